# revision 8
# baseline (speedup 1.0000x reference)
"""Bahdanau-attention Trainium2 kernel (8 NeuronCores, data-parallel over batch).

Problem shapes (full): decoder_hidden [64,1024], encoder_outputs [64,2048,1024],
W1_w/W2_w [1024,1024], W1_b/W2_b/v_w [1024].
Returns (context [64,1024], attn_weights [64,2048]) as float32, matching:
    enc_proj = enc @ W1 + b1
    dec_proj = dec @ W2 + b2
    e = tanh(enc_proj + dec_proj[:, None, :])
    scores = e @ v ; attn = softmax(scores, axis=S) ; context = attn @ enc

Sharding: batch 64 -> 8 per core, weights replicated, no collectives.

Per-core plan (per batch of the 8):
  - one SWDGE cast-DMA converts enc f32 -> bf16 DRAM staging (row order)
  - two xbar-transpose DMAs load encT [128 h_in-part, 8 h_chunk, tok]
    (3D transpose output folds rows as h_in = kc*128 + p)
  - W1 matmuls (bf16, W1 stationary), ACT fuses (dec@W2+b1+b2) bias + tanh
  - scores via PE v-dot (M=1), exact max-subtracted softmax (ACT exp with
    fused denominator), exp weights sprayed to [128,16] via a DRAM bounce
  - context via PE (exp-weight columns stationary, natural bf16 tiles
    reloaded from staging as the moving operand)
"""

import sys

for _p in ("/opt/trn_rl_repo", "/opt/pypackages"):
    if _p not in sys.path:
        sys.path.insert(0, _p)

import numpy as np

import concourse.bass as bass  # noqa: E402
import concourse.mybir as mybir  # noqa: E402
import concourse.tile as tile  # noqa: E402
from concourse import bacc  # noqa: E402
from concourse.bass_utils import run_bass_kernel_spmd  # noqa: E402

F32 = mybir.dt.float32
BF16 = mybir.dt.bfloat16
AF = mybir.ActivationFunctionType
ALU = mybir.AluOpType

NCORES = 8
B = 8        # batches per core
S = 2048
H = 1024
NT = 512     # tokens per inner compute tile
NNT = S // NT          # 4 inner tiles per batch
HCH = H // 128         # 8 h-chunks
TPB = S // 128         # 16 token-chunks per batch
SH = S // 2            # tokens per transpose half


def build_kernel():
    nc = bacc.Bacc("TRN2", target_bir_lowering=False, debug=False)

    enc = nc.dram_tensor("encoder_outputs", [B, S, H], F32, kind="ExternalInput").ap()
    dec = nc.dram_tensor("decoder_hidden", [B, H], F32, kind="ExternalInput").ap()
    w1 = nc.dram_tensor("W1_w", [H, H], F32, kind="ExternalInput").ap()
    b1 = nc.dram_tensor("W1_b", [H], F32, kind="ExternalInput").ap()
    w2 = nc.dram_tensor("W2_w", [H, H], F32, kind="ExternalInput").ap()
    b2 = nc.dram_tensor("W2_b", [H], F32, kind="ExternalInput").ap()
    vw = nc.dram_tensor("v_w", [H], F32, kind="ExternalInput").ap()
    ctx_out = nc.dram_tensor("context", [B, H], F32, kind="ExternalOutput").ap()
    attn_out = nc.dram_tensor("attn", [B, S], F32, kind="ExternalOutput").ap()

    with tile.TileContext(nc) as tc:
        with (
            tc.tile_pool(name="singles", bufs=1) as singles,
            tc.tile_pool(name="nat", bufs=8) as nat_pool,
            tc.tile_pool(name="enct", bufs=3) as enct_pool,
            tc.tile_pool(name="esb", bufs=2) as esb_pool,
            tc.tile_pool(name="rows", bufs=2) as row_pool,
            tc.tile_pool(name="smalls", bufs=4) as small_pool,
            tc.tile_pool(name="stage", bufs=2, space="DRAM") as stage_pool,
            tc.tile_pool(name="ps_e", bufs=2, space="PSUM") as ps_e,
            tc.tile_pool(name="ps_s", bufs=2, space="PSUM") as ps_s,
            tc.tile_pool(name="ps_c", bufs=2, space="PSUM") as ps_c,
        ):
            # ---------------- weights / constants (one-time) ----------------
            # W1 as lhsT chunks matching the folded transpose layout:
            # encT partition p / chunk kc holds h_in = kc*128 + p, so
            # w1_sb[p, kc, m] = W1[kc*128 + p, m]
            w1_sb = singles.tile([128, HCH, H], BF16)
            nc.gpsimd.dma_start(
                out=w1_sb, in_=w1.rearrange("(kc p) m -> p kc m", p=128)
            )
            # W2 in plain chunk-major layout (h_in = kc*128 + p)
            w2_sb = singles.tile([128, HCH, H], BF16)
            nc.gpsimd.dma_start(
                out=w2_sb, in_=w2.rearrange("(kc p) m -> p kc m", p=128)
            )
            # v / biases indexed by h_out = c*128 + p (matmul M column order)
            v_sb = singles.tile([128, HCH], BF16)
            nc.gpsimd.dma_start(out=v_sb, in_=vw.rearrange("(c p) -> p c", p=128))
            b1_sb = singles.tile([128, HCH], F32)
            nc.gpsimd.dma_start(out=b1_sb, in_=b1.rearrange("(c p) -> p c", p=128))
            b2_sb = singles.tile([128, HCH], F32)
            nc.gpsimd.dma_start(out=b2_sb, in_=b2.rearrange("(c p) -> p c", p=128))
            bias_sb = singles.tile([128, HCH], F32)
            nc.vector.tensor_add(bias_sb, b1_sb, b2_sb)
            # dec^T chunks: dect[p, c, b] = dec[b, c*128+p]
            dect_f = singles.tile([128, HCH, B], F32)
            dec_r = dec.rearrange("b (c p) -> p c b", p=128)
            for c in range(HCH):
                nc.gpsimd.dma_start(out=dect_f[:, c, :], in_=dec_r[:, c, :])
            dect = singles.tile([128, HCH, B], BF16)
            nc.vector.tensor_copy(dect, dect_f)

            # ---------------- D^T = (dec @ W2 + b1 + b2)^T  [128, HCH, B] ----
            d_sb = singles.tile([128, HCH, B], F32)
            for mc in range(HCH):
                ps_d = ps_e.tile([128, NT], F32, tag="epsum")
                for kc in range(HCH):
                    nc.tensor.matmul(
                        ps_d[:, :B],
                        lhsT=w2_sb[:, kc, mc * 128:(mc + 1) * 128],
                        rhs=dect[:, kc, :],
                        start=(kc == 0),
                        stop=(kc == HCH - 1),
                    )
                nc.vector.tensor_scalar(
                    d_sb[:, mc, :], ps_d[:, :B], bias_sb[:, mc:mc + 1], None, ALU.add
                )

            # ---------------- main loop over batches ----------------
            for b in range(B):
                # bf16 staging copy of this batch in DRAM (token-order rows)
                stg = stage_pool.tile([S, H], BF16)
                nc.gpsimd.dma_start(out=stg, in_=enc[b])
                # two xbar transposes: encT[p, kc, tok] = enc[b, tok, p*8+kc]
                enct0 = enct_pool.tile([128, HCH, SH], BF16, tag="enct")
                nc.sync.dma_start_transpose(out=enct0, in_=stg[:SH, :])
                enct1 = enct_pool.tile([128, HCH, SH], BF16, tag="enct")
                nc.sync.dma_start_transpose(out=enct1, in_=stg[SH:, :])
                enct_half = [enct0, enct1]

                scores_row = row_pool.tile([1, S], F32, tag="scores")
                for nt in range(NNT):
                    enct = enct_half[nt // 2]
                    n0 = (nt % 2) * NT
                    # W1 matmuls + fused bias/tanh
                    e_sb = esb_pool.tile([128, HCH, NT], BF16)
                    for mc in range(HCH):
                        ps = ps_e.tile([128, NT], F32, tag="epsum")
                        for kc in range(HCH):
                            nc.tensor.matmul(
                                ps,
                                lhsT=w1_sb[:, kc, mc * 128:(mc + 1) * 128],
                                rhs=enct[:, kc, n0:n0 + NT],
                                start=(kc == 0),
                                stop=(kc == HCH - 1),
                            )
                        nc.scalar.activation(
                            out=e_sb[:, mc, :], in_=ps, func=AF.Tanh,
                            bias=d_sb[:, mc, b:b + 1],
                        )
                    # scores for this tile: [1, NT]
                    ps_sc = ps_s.tile([1, NT], F32, tag="spsum")
                    for hc in range(HCH):
                        nc.tensor.matmul(
                            ps_sc,
                            lhsT=v_sb[:, hc:hc + 1],
                            rhs=e_sb[:, hc, :],
                            start=(hc == 0),
                            stop=(hc == HCH - 1),
                        )
                    nc.vector.tensor_copy(
                        scores_row[:, nt * NT:(nt + 1) * NT], ps_sc
                    )

                # softmax pieces: exact, max-subtracted
                neg_max = small_pool.tile([1, 1], F32, tag="negmax")
                nc.vector.tensor_reduce(
                    neg_max, scores_row, mybir.AxisListType.X, ALU.max, negate=True
                )
                exp_row = row_pool.tile([1, S], F32, tag="exps")
                den = small_pool.tile([1, 1], F32, tag="den")
                nc.scalar.activation(
                    out=exp_row, in_=scores_row, func=AF.Exp, bias=neg_max,
                    accum_out=den,
                )
                rden = small_pool.tile([1, 1], F32, tag="rden")
                nc.vector.reciprocal(rden, den)

                # spray exp weights across partitions: wt[p, c] = w[c*128+p]
                # (via DRAM bounce — SBUF partition dim can't absorb free bytes)
                wrow_d = stage_pool.tile([S], F32, tag="wrow")
                nc.sync.dma_start(out=wrow_d, in_=exp_row)
                wt_f = small_pool.tile([128, TPB], F32, tag="wtf")
                nc.sync.dma_start(
                    out=wt_f, in_=wrow_d.rearrange("(c p) -> p c", p=128)
                )
                wt = small_pool.tile([128, TPB], BF16, tag="wtb")
                nc.vector.tensor_copy(wt, wt_f)

                # context: ctx[h] = sum_tok w[tok]*enc[tok, h]; natural bf16
                # chunks reloaded from staging, exp-weight column stationary
                ps_cx0 = ps_c.tile([1, NT], F32, tag="cpsum0")
                ps_cx1 = ps_c.tile([1, NT], F32, tag="cpsum1")
                ps_cx = [ps_cx0, ps_cx1]
                for c in range(TPB):
                    natc = nat_pool.tile([128, H], BF16, tag="natc")
                    nc.sync.dma_start(
                        out=natc, in_=stg[c * 128:(c + 1) * 128, :]
                    )
                    for nh in range(2):
                        nc.tensor.matmul(
                            ps_cx[nh],
                            lhsT=wt[:, c:c + 1],
                            rhs=natc[:, nh * NT:(nh + 1) * NT],
                            start=(c == 0),
                            stop=(c == TPB - 1),
                        )
                ctx_sb = small_pool.tile([1, H], F32, tag="ctx")
                for nh in range(2):
                    nc.vector.tensor_scalar(
                        ctx_sb[:, nh * NT:(nh + 1) * NT], ps_cx[nh], rden, None,
                        ALU.mult,
                    )
                # normalize attn in place (after wrow_d store, Tile orders WAR)
                nc.vector.tensor_scalar(exp_row, exp_row, rden, None, ALU.mult)

                nc.sync.dma_start(out=ctx_out[b:b + 1, :], in_=ctx_sb)
                nc.sync.dma_start(out=attn_out[b:b + 1, :], in_=exp_row)

    nc.compile()
    return nc


_NC = None


def _get_nc():
    global _NC
    if _NC is None:
        _NC = build_kernel()
    return _NC


def kernel(decoder_hidden, encoder_outputs, W1_w, W1_b, W2_w, W2_b, v_w):
    nc = _get_nc()

    def fb(x):
        return np.ascontiguousarray(np.asarray(x, dtype=np.float32))

    in_maps = []
    for i in range(NCORES):
        lo, hi = i * B, (i + 1) * B
        in_maps.append({
            "encoder_outputs": fb(encoder_outputs[lo:hi]),
            "decoder_hidden": fb(decoder_hidden[lo:hi]),
            "W1_w": fb(W1_w),
            "W1_b": fb(W1_b),
            "W2_w": fb(W2_w),
            "W2_b": fb(W2_b),
            "v_w": fb(v_w),
        })
    res = run_bass_kernel_spmd(nc, in_maps, core_ids=list(range(NCORES)))
    results = res.results
    context = np.concatenate([r["context"] for r in results], axis=0)
    attn = np.concatenate([r["attn"] for r in results], axis=0)
    return context, attn


# revision 38
# speedup vs baseline: 11.2062x; 11.2062x over previous
"""Bahdanau-attention Trainium2 kernel (8 NeuronCores, data-parallel over batch).

Problem shapes (full): decoder_hidden [64,1024], encoder_outputs [64,2048,1024],
W1_w/W2_w [1024,1024], W1_b/W2_b/v_w [1024].
Returns (context [64,1024], attn_weights [64,2048]) as float32, matching:
    enc_proj = enc @ W1 + b1
    dec_proj = dec @ W2 + b2
    e = tanh(enc_proj + dec_proj[:, None, :])
    scores = e @ v ; attn = softmax(scores, axis=S) ; context = attn @ enc

Sharding: batch 64 -> 8 per core, weights replicated, no collectives.

Per-core plan (per batch of the 8):
  - SWDGE cast-DMAs convert enc f32 -> bf16 DRAM staging (row order), one
    per half-batch so the transpose pipeline starts early
  - two xbar-transpose DMAs load encT [128 h_in-part, 8 h_chunk, tok]
    (3D transpose output folds rows as h_in = kc*128 + p)
  - W1 matmuls (bf16, W1 stationary), ACT fuses (dec@W2+b1+b2) bias + tanh
  - scores via PE v-dot (M=1), exact max-subtracted softmax (ACT exp with
    fused denominator)
  - context on DVE: tensor_tensor_reduce of encT with the exp weights
    broadcast across partitions (via a DRAM bounce row)
"""

import sys

for _p in ("/opt/trn_rl_repo", "/opt/pypackages"):
    if _p not in sys.path:
        sys.path.insert(0, _p)

import ml_dtypes
import numpy as np

import concourse.bass as bass  # noqa: E402
import concourse.mybir as mybir  # noqa: E402
import concourse.tile as tile  # noqa: E402
from concourse import bacc  # noqa: E402
from concourse.bass_utils import run_bass_kernel_spmd  # noqa: E402

F32 = mybir.dt.float32
BF16 = mybir.dt.bfloat16
AF = mybir.ActivationFunctionType
ALU = mybir.AluOpType

NCORES = 8
B = 8        # batches per core
S = 2048
H = 1024
NT = 512     # tokens per inner compute tile
NNT = S // NT          # 4 inner tiles per batch
HCH = H // 128         # 8 h-chunks
SH = S // 2            # tokens per transpose half


def build_kernel():
    nc = bacc.Bacc("TRN2", target_bir_lowering=False, debug=False)

    enc = nc.dram_tensor("encoder_outputs", [B, S, H], F32, kind="ExternalInput").ap()
    dec = nc.dram_tensor("decoder_hidden", [B, H], F32, kind="ExternalInput").ap()
    w1 = nc.dram_tensor("W1_bf", [128, HCH * H], BF16, kind="ExternalInput").ap()
    b1 = nc.dram_tensor("W1_b", [H], F32, kind="ExternalInput").ap()
    w2 = nc.dram_tensor("W2_bf", [128, HCH * H], BF16, kind="ExternalInput").ap()
    b2 = nc.dram_tensor("W2_b", [H], F32, kind="ExternalInput").ap()
    vw = nc.dram_tensor("v_w", [H], F32, kind="ExternalInput").ap()
    ctx_out = nc.dram_tensor("context", [B, H], F32, kind="ExternalOutput").ap()
    attn_out = nc.dram_tensor("attn", [B, S], F32, kind="ExternalOutput").ap()

    with tile.TileContext(nc) as tc:
        with (
            tc.tile_pool(name="singles", bufs=1) as singles,
            tc.tile_pool(name="enct", bufs=4) as enct_pool,
            tc.tile_pool(name="esb", bufs=2) as esb_pool,
            tc.tile_pool(name="rows", bufs=2) as row_pool,
            tc.tile_pool(name="wb", bufs=2) as wb_pool,
            tc.tile_pool(name="nat", bufs=1) as nat_pool,
            tc.tile_pool(name="scr", bufs=1) as scr_pool,
            tc.tile_pool(name="ctxr", bufs=1) as ctxr_pool,
            tc.tile_pool(name="smalls", bufs=4) as small_pool,
            tc.tile_pool(name="stage", bufs=8, space="DRAM") as stage_pool,
            tc.tile_pool(name="ps_e", bufs=6, space="PSUM") as ps_e,
            tc.tile_pool(name="ps_s", bufs=2, space="PSUM") as ps_s,
        ):
            # SWDGE queue order at startup: W1 (bf16, host-packed) first
            # since it gates every matmul, then batch-0 staging quarters
            # (each unblocks one transpose), W2 (gates D / first tanh),
            # then batches 1-2 pre-staged. w_sb[p, kc, m] = W[kc*128+p, m]
            # matches the transpose fold h_in = kc*128 + p.
            w1_sb = singles.tile([128, HCH, H], BF16)
            nc.gpsimd.dma_start(out=w1_sb, in_=w1)
            stq0 = stage_pool.tile([SH, H], BF16, tag="stg")
            nc.gpsimd.dma_start(out=stq0[:NT, :], in_=enc[0, :NT, :])
            stq1 = stage_pool.tile([SH, H], BF16, tag="stg")
            nc.gpsimd.dma_start(out=stq1[:NT, :], in_=enc[0, NT:SH, :])
            w2_sb = singles.tile([128, HCH, H], BF16)
            nc.gpsimd.dma_start(out=w2_sb, in_=w2)
            stq2 = stage_pool.tile([SH, H], BF16, tag="stg")
            nc.gpsimd.dma_start(out=stq2[:NT, :], in_=enc[0, SH:SH + NT, :])
            stq3 = stage_pool.tile([SH, H], BF16, tag="stg")
            nc.gpsimd.dma_start(out=stq3[:NT, :], in_=enc[0, SH + NT:, :])
            stg0_quarters = [stq0, stq1, stq2, stq3]

            def stage_batch(b):
                stgh0 = stage_pool.tile([SH, H], BF16, tag="stg")
                nc.gpsimd.dma_start(out=stgh0, in_=enc[b, :SH, :])
                stgh1 = stage_pool.tile([SH, H], BF16, tag="stg")
                nc.gpsimd.dma_start(out=stgh1, in_=enc[b, SH:, :])
                return stgh0, stgh1

            staged = {1: stage_batch(1), 2: stage_batch(2)}
            # v / biases indexed by h_out = c*128 + p (matmul M column order);
            # f32 loads on the HWDGE queue, converts on DVE
            v_f = singles.tile([128, HCH], F32)
            nc.sync.dma_start(out=v_f, in_=vw.rearrange("(c p) -> p c", p=128))
            v_sb = singles.tile([128, HCH], BF16)
            nc.vector.tensor_copy(v_sb, v_f)
            b1_sb = singles.tile([128, HCH], F32)
            nc.sync.dma_start(out=b1_sb, in_=b1.rearrange("(c p) -> p c", p=128))
            b2_sb = singles.tile([128, HCH], F32)
            nc.sync.dma_start(out=b2_sb, in_=b2.rearrange("(c p) -> p c", p=128))
            bias_sb = singles.tile([128, HCH], F32)
            nc.vector.tensor_add(bias_sb, b1_sb, b2_sb)
            # dec^T chunks: dect[p, c, b] = dec[b, c*128+p]
            dect_f = singles.tile([128, HCH, B], F32)
            dec_r = dec.rearrange("b (c p) -> p c b", p=128)
            for c in range(HCH):
                nc.sync.dma_start(out=dect_f[:, c, :], in_=dec_r[:, c, :])
            dect = singles.tile([128, HCH, B], BF16)
            nc.vector.tensor_copy(dect, dect_f)
            ones_col = singles.tile([128, 1], BF16)
            nc.vector.memset(ones_col, 1.0)

            # ---------------- D^T = (dec @ W2 + b1 + b2)^T  [128, HCH, B] ----
            d_sb = singles.tile([128, HCH, B], F32)
            for mc in range(HCH):
                ps_d = ps_e.tile([128, NT], F32, tag="epsum")
                for kc in range(HCH):
                    nc.tensor.matmul(
                        ps_d[:, :B],
                        lhsT=w2_sb[:, kc, mc * 128:(mc + 1) * 128],
                        rhs=dect[:, kc, :],
                        start=(kc == 0),
                        stop=(kc == HCH - 1),
                    )
                nc.vector.tensor_scalar(
                    d_sb[:, mc, :], ps_d[:, :B], bias_sb[:, mc:mc + 1], None, ALU.add
                )

            # ---------------- main loop over batches ----------------
            for b in range(B):
                # xbar transposes: encT[p, kc, tok] = enc[b, tok, kc*128+p]
                # (batch 0 in quarters: its first compute tile unblocks as
                # soon as the first quarter cast + transpose land)
                enct0 = enct_pool.tile([128, HCH, SH], BF16, tag="enct")
                enct1 = enct_pool.tile([128, HCH, SH], BF16, tag="enct")
                enct_half = [enct0, enct1]
                if b == 0:
                    for q in range(4):
                        nc.sync.dma_start_transpose(
                            out=enct_half[q // 2][:, :, (q % 2) * NT:(q % 2 + 1) * NT],
                            in_=stg0_quarters[q][:NT, :],
                        )
                else:
                    stgh = staged.pop(b)
                    nc.sync.dma_start_transpose(out=enct0, in_=stgh[0])
                    nc.sync.dma_start_transpose(out=enct1, in_=stgh[1])
                if b + 3 < B and b + 3 not in staged:
                    staged[b + 3] = stage_batch(b + 3)

                # the last batch computes its context on PE (otherwise PE
                # idles through the final softmax+reduce tail): prefetch
                # natural-layout chunks from staging during the compute
                nat_all = None
                if b == B - 1:
                    # nat_all[p, c, h] = enc[b, 16p + c, h] (one contiguous
                    # descriptor per partition from each staging half)
                    nat_all = nat_pool.tile([128, S // 128, H], BF16, tag="natc")
                    nc.gpsimd.dma_start(
                        out=nat_all[:64, :, :],
                        in_=stgh[0].rearrange("(p c) h -> p c h", c=S // 128),
                    )
                    nc.gpsimd.dma_start(
                        out=nat_all[64:, :, :],
                        in_=stgh[1].rearrange("(p c) h -> p c h", c=S // 128),
                    )

                scores_row = row_pool.tile([1, S], F32, tag="scores")
                for nt in range(NNT):
                    enct = enct_half[nt // 2]
                    n0 = (nt % 2) * NT
                    # W1 matmuls + fused bias/tanh
                    e_sb = esb_pool.tile([128, HCH, NT], BF16)
                    for mc in range(HCH):
                        ps = ps_e.tile([128, NT], F32, tag="epsum")
                        for kc in range(HCH):
                            nc.tensor.matmul(
                                ps,
                                lhsT=w1_sb[:, kc, mc * 128:(mc + 1) * 128],
                                rhs=enct[:, kc, n0:n0 + NT],
                                start=(kc == 0),
                                stop=(kc == HCH - 1),
                            )
                        nc.scalar.activation(
                            out=e_sb[:, mc, :], in_=ps, func=AF.Tanh,
                            bias=d_sb[:, mc, b:b + 1],
                        )
                    # scores: fold v into the chunks on DVE (FMA chain), then
                    # one cross-partition sum on PE via a ones column
                    g_acc = esb_pool.tile([128, NT], BF16, tag="gacc")
                    nc.vector.tensor_scalar(
                        g_acc, e_sb[:, 0, :], v_f[:, 0:1], None, ALU.mult
                    )
                    for mc in range(1, HCH):
                        nc.vector.scalar_tensor_tensor(
                            out=g_acc, in0=e_sb[:, mc, :],
                            scalar=v_f[:, mc:mc + 1], in1=g_acc,
                            op0=ALU.mult, op1=ALU.add,
                        )
                    ps_sc = ps_s.tile([1, NT], F32, tag="spsum")
                    nc.tensor.matmul(
                        ps_sc, lhsT=ones_col, rhs=g_acc, start=True, stop=True
                    )
                    nc.vector.tensor_copy(
                        scores_row[:, nt * NT:(nt + 1) * NT], ps_sc
                    )

                # softmax pieces: exact, max-subtracted
                neg_max = small_pool.tile([1, 1], F32, tag="negmax")
                nc.vector.tensor_reduce(
                    neg_max, scores_row, mybir.AxisListType.X, ALU.max, negate=True
                )
                exp_row = row_pool.tile([1, S], F32, tag="exps")
                den = small_pool.tile([1, 1], F32, tag="den")
                nc.scalar.activation(
                    out=exp_row, in_=scores_row, func=AF.Exp, bias=neg_max,
                    accum_out=den,
                )
                rden = small_pool.tile([1, 1], F32, tag="rden")
                nc.vector.reciprocal(rden, den)
                # normalize in place: exp_row becomes attn weights
                nc.vector.tensor_scalar(exp_row, exp_row, rden, None, ALU.mult)

                wrow_d = stage_pool.tile([S], F32, tag="wrow")
                nc.gpsimd.dma_start(out=wrow_d, in_=exp_row)

                if b < B - 1:
                    # broadcast attn weights to all partitions via the DRAM
                    # bounce (SWDGE: replication + f32->bf16 cast in one DMA),
                    # then context on DVE:
                    #   ctx[kc*128+p] = sum_tok encT[p,kc,tok]*w[tok]
                    wrow_bc = bass.AP(
                        tensor=wrow_d.tensor, offset=wrow_d.offset,
                        ap=[[0, 128], *wrow_d.ap],
                    )
                    wb = wb_pool.tile([128, S], BF16, tag="wb")
                    nc.gpsimd.dma_start(out=wb, in_=wrow_bc)

                    ttr_out = scr_pool.tile([128, SH], BF16, tag="ttro")
                    ctx_h0 = small_pool.tile([128, HCH], F32, tag="ctx0")
                    ctx_h1 = small_pool.tile([128, HCH], F32, tag="ctx1")
                    ctx_h = [ctx_h0, ctx_h1]
                    for hf in range(2):
                        for kc in range(HCH):
                            nc.vector.affine_mul_reduce(
                                out=ttr_out,
                                accum_out=ctx_h[hf][:, kc:kc + 1],
                                in0=enct_half[hf][:, kc, :],
                                in1=wb[:, hf * SH:(hf + 1) * SH],
                                scale=1.0,
                                bias=0.0,
                            )
                    ctx_sb = small_pool.tile([128, HCH], F32, tag="ctxs")
                    nc.vector.tensor_add(ctx_sb, ctx_h[0], ctx_h[1])

                    nc.gpsimd.dma_start(
                        out=ctx_out[b].rearrange("(c p) -> p c", p=128),
                        in_=ctx_sb,
                    )
                else:
                    # last batch: context on PE with the normalized weights
                    # sprayed to one column per 128-token chunk
                    wt_f = small_pool.tile([128, S // 128], F32, tag="wtf")
                    nc.gpsimd.dma_start(
                        out=wt_f, in_=wrow_d.rearrange("(p c) -> p c", p=128)
                    )
                    wt = small_pool.tile([128, S // 128], BF16, tag="wtb")
                    nc.vector.tensor_copy(wt, wt_f)
                    ps_cx0 = ps_s.tile([1, NT], F32, tag="spsum")
                    ps_cx1 = ps_s.tile([1, NT], F32, tag="spsum")
                    ps_cx = [ps_cx0, ps_cx1]
                    for c in range(S // 128):
                        for nh in range(2):
                            nc.tensor.matmul(
                                ps_cx[nh],
                                lhsT=wt[:, c:c + 1],
                                rhs=nat_all[:, c, nh * NT:(nh + 1) * NT],
                                start=(c == 0),
                                stop=(c == S // 128 - 1),
                            )
                    ctx_row = ctxr_pool.tile([1, H], F32, tag="ctxr")
                    for nh in range(2):
                        nc.vector.tensor_copy(
                            ctx_row[:, nh * NT:(nh + 1) * NT], ps_cx[nh]
                        )
                    nc.gpsimd.dma_start(out=ctx_out[b:b + 1, :], in_=ctx_row)
                nc.gpsimd.dma_start(out=attn_out[b:b + 1, :], in_=exp_row)

    nc.compile()
    return nc


_NC = None


def _get_nc():
    global _NC
    if _NC is None:
        _NC = build_kernel()
    return _NC


def make_in_maps(decoder_hidden, encoder_outputs, W1_w, W1_b, W2_w, W2_b, v_w):
    def fb(x):
        return np.ascontiguousarray(np.asarray(x, dtype=np.float32))

    def pack_w(w):
        wb = np.asarray(w, np.float32).astype(ml_dtypes.bfloat16)
        # [kc*128+p, m] -> [p, kc*H + m]
        return np.ascontiguousarray(
            wb.reshape(HCH, 128, H).transpose(1, 0, 2).reshape(128, HCH * H))

    w1bf = pack_w(W1_w)
    w2bf = pack_w(W2_w)

    in_maps = []
    for i in range(NCORES):
        lo, hi = i * B, (i + 1) * B
        in_maps.append({
            "encoder_outputs": fb(encoder_outputs[lo:hi]),
            "decoder_hidden": fb(decoder_hidden[lo:hi]),
            "W1_bf": w1bf,
            "W1_b": fb(W1_b),
            "W2_bf": w2bf,
            "W2_b": fb(W2_b),
            "v_w": fb(v_w),
        })
    return in_maps


def kernel(decoder_hidden, encoder_outputs, W1_w, W1_b, W2_w, W2_b, v_w):
    nc = _get_nc()
    in_maps = make_in_maps(decoder_hidden, encoder_outputs, W1_w, W1_b,
                           W2_w, W2_b, v_w)
    res = run_bass_kernel_spmd(nc, in_maps, core_ids=list(range(NCORES)))
    results = res.results
    context = np.concatenate([r["context"] for r in results], axis=0)
    attn = np.concatenate([r["attn"] for r in results], axis=0)
    return context, attn
